# revision 22
# baseline (speedup 1.0000x reference)
"""Trainium2 Bass kernel for nn_HeatEquation1D.

The reference applies a fixed 62x62 Crank-Nicolson step matrix 100 times to
u0[:, 1:-1] via lax.scan, then zero-pads the boundary columns.  Algebraically
that whole scan is a single matmul:

    out = u0 @ W64,   W64[1:63, 1:63] = (step_matrix^100).T,  zero elsewhere

(the zero rows/cols of W64 implement both the dropped boundary inputs and the
zero Dirichlet boundary outputs).  W64 is computed on the host in float64.

Device kernel (per core, pure data parallel over 8 cores):
  - u shard (65536, 64) f32 moves in SUPER_ROWS-row super-blocks; each
    partition holds SUPER_ROWS/128 consecutive rows, so one dma_start is a
    single large contiguous chunk per partition (amortizes the fixed
    per-DMA overheads).  The input DMA is SWDGE and casts f32->bf16
    inline, so every PE operand is bf16 (1 cyc/row + fast weight load).
  - Compute runs in 2048-row groups (fits PSUM):
      * 8 PE transposes (bf16): X chunk [128, 2rows x 64feat] -> T1 in PSUM.
      * 1 DVE copy PSUM->SBUF bf16 (all-2-byte operands -> DVE 2x mode).
      * 8 PE matmuls: stationary = T1 chunk, moving = BD where
        BD = block_diag(W64, W64) (128x128) in bf16.  Because T1 chunk
        columns are (row-pair, feature) interleaved, BD applies W64 to each
        row of the pair and the result lands batch-major in fp32 PSUM --
        no second transpose.
      * 1 ACT copy PSUM->SBUF fp32 (output copy on the Scalar engine so the
        two mandatory PSUM evacuations ride different engines).
  - One contiguous HWDGE dma_start out per super-block.

Measured 8-core-concurrent on trn2: ~108 us/core vs a 95 us pure-DMA
floor (33.55 MB at the ~352 GB/s HBM-per-core limit); PE/DVE/ACT hide
underneath.  Numerics: single bf16 rounding of data and matrix,
rel_err ~2.3e-3 (tolerance 2e-2).
"""

import numpy as np

BATCH = 524288
NX = 64
N_INNER = NX - 2
NUM_STEPS = 100
N_CORES = 8
ROWS_PER_CORE = BATCH // N_CORES           # 65536
P = 128

SUPER_ROWS = 16384                         # rows per DMA super-block (4 MiB)
GROUP_ROWS = 2048                          # rows per compute group (PSUM bound)

# Set by callers that want a profile; results object stashed in LAST_RESULTS.
TRACE = False
LAST_RESULTS = None

_NC_CACHE = {}


def _build_nc(reps=1, dma_only=False, super_rows=SUPER_ROWS,
              group_rows=GROUP_ROWS, split_bd=False, cast="dma"):
    """reps>1 wraps the whole pass in a hardware For_i loop (for benching).

    cast: how u f32 becomes bf16 before the PE transposes (bf16 operands
    run 1 cyc/row on the PE and get fast weight load):
      "engine" - plain HWDGE f32 input DMA, GPSIMD casts each group's
                 slice to bf16 (the SWDGE cast-DMA measured ~11% below
                 DMA line rate at 8 cores; GPSIMD is otherwise idle).
      "dma"    - SWDGE input DMA casts inline.
      "none"   - stay f32 into the transposes (PE 2 cyc/row, no FWL).
    """
    from concourse import bacc, mybir
    from concourse.tile import TileContext

    nc = bacc.Bacc("TRN2", target_bir_lowering=False, debug=False)
    f32 = mybir.dt.float32
    bf16 = mybir.dt.bfloat16
    if dma_only:
        cast = "none"
    assert cast in ("engine", "dma", "none")
    x_dt = bf16 if cast == "dma" else f32      # dtype of the DMA-in tile
    t_dt = f32 if cast == "none" else bf16     # dtype feeding the transposes
    in_dma = nc.gpsimd.dma_start if cast == "dma" else nc.sync.dma_start

    n_super = ROWS_PER_CORE // super_rows
    rpp_s = super_rows // P                 # rows per partition per super tile
    groups = super_rows // group_rows       # compute groups per super-block
    rpp_g = group_rows // P                 # rows per partition per group
    ch = rpp_g // 2                         # chunks of 128 cols per group

    u = nc.dram_tensor("u", [ROWS_PER_CORE, NX], f32, kind="ExternalInput")
    bdh_d = nc.dram_tensor("bd_hi", [P, P], bf16, kind="ExternalInput")
    bdl_d = nc.dram_tensor("bd_lo", [P, P], bf16, kind="ExternalInput")
    id_d = nc.dram_tensor("ident", [P, P], t_dt, kind="ExternalInput")
    out = nc.dram_tensor("out", [ROWS_PER_CORE, NX], f32, kind="ExternalOutput")

    u_r = u.rearrange("(nb p r) f -> nb p r f", p=P, r=rpp_s)
    out_r = out.rearrange("(nb p r) f -> nb p r f", p=P, r=rpp_s)

    with TileContext(nc) as tc:
        with (
            tc.tile_pool(name="consts", bufs=1) as cpool,
            tc.tile_pool(name="xin", bufs=3) as xpool,
            tc.tile_pool(name="xb16", bufs=3) as bpool,
            tc.tile_pool(name="t1s", bufs=3) as tpool,
            tc.tile_pool(name="yout", bufs=3) as ypool,
            tc.tile_pool(name="ps_t", bufs=2, space="PSUM") as pst,
            tc.tile_pool(name="ps_y", bufs=2, space="PSUM") as psy,
        ):
            bdh_s = cpool.tile([P, P], bf16)
            bdl_s = cpool.tile([P, P], bf16)
            id_s = cpool.tile([P, P], t_dt)
            nc.sync.dma_start(out=bdh_s[:], in_=bdh_d[:])
            nc.sync.dma_start(out=bdl_s[:], in_=bdl_d[:])
            nc.sync.dma_start(out=id_s[:], in_=id_d[:])

            def one_pass():
                for nb in range(n_super):
                    x = xpool.tile([P, rpp_s, NX], x_dt)
                    in_dma(out=x[:], in_=u_r[nb])

                    if dma_only:
                        nc.sync.dma_start(out=out_r[nb], in_=x[:])
                        continue

                    y = ypool.tile([P, rpp_s, NX], f32)
                    for g in range(groups):
                        r0 = g * rpp_g
                        if cast == "engine":
                            src = bpool.tile([P, rpp_g, NX], bf16)
                            nc.gpsimd.tensor_copy(
                                out=src[:], in_=x[:, r0 : r0 + rpp_g, :]
                            )
                            s0 = 0
                        else:
                            src, s0 = x, r0
                        t1p = pst.tile([P, ch, P], t_dt)
                        for c in range(ch):
                            nc.tensor.transpose(
                                t1p[:, c],
                                src[:, s0 + 2 * c : s0 + 2 * c + 2, :],
                                id_s[:],
                            )
                        t1s = tpool.tile([P, ch, P], bf16)
                        nc.vector.tensor_copy(out=t1s[:], in_=t1p[:])

                        yp = psy.tile([P, ch, P], f32)
                        for c in range(ch):
                            nc.tensor.matmul(
                                yp[:, c], t1s[:, c], bdh_s[:],
                                start=True, stop=not split_bd,
                            )
                            if split_bd:
                                nc.tensor.matmul(
                                    yp[:, c], t1s[:, c], bdl_s[:],
                                    start=False, stop=True,
                                )
                        nc.scalar.copy(
                            out=y[:, r0 : r0 + rpp_g, :], in_=yp[:]
                        )
                    nc.sync.dma_start(out=out_r[nb], in_=y[:])

            if reps == 1:
                one_pass()
            else:
                with tc.For_i(0, reps, 1,
                              hint_engines=(mybir.EngineType.PE,)):
                    one_pass()

    nc.compile()
    return nc


def _host_matrices(step_matrix):
    import ml_dtypes

    m = np.asarray(step_matrix, dtype=np.float64)
    w_inner = np.linalg.matrix_power(m, NUM_STEPS).T  # right-multiplier, f64
    w64 = np.zeros((NX, NX), dtype=np.float64)
    w64[1 : NX - 1, 1 : NX - 1] = w_inner
    bd = np.zeros((P, P), dtype=np.float64)
    bd[:NX, :NX] = w64
    bd[NX:, NX:] = w64
    bd_hi = bd.astype(ml_dtypes.bfloat16)
    bd_lo = (bd - bd_hi.astype(np.float64)).astype(ml_dtypes.bfloat16)
    return bd_hi, bd_lo


def _const_inputs(step_matrix, cast="dma"):
    import ml_dtypes

    bd_hi, bd_lo = _host_matrices(step_matrix)
    id_dt = np.float32 if cast == "none" else ml_dtypes.bfloat16
    ident = np.eye(P, dtype=id_dt)
    return {"bd_hi": bd_hi, "bd_lo": bd_lo, "ident": ident}


def kernel(u0, step_matrix):
    global LAST_RESULTS
    from concourse.bass_utils import run_bass_kernel_spmd

    u0 = np.ascontiguousarray(np.asarray(u0, dtype=np.float32))
    assert u0.shape == (BATCH, NX), u0.shape

    consts = _const_inputs(step_matrix)

    if "nc" not in _NC_CACHE:
        _NC_CACHE["nc"] = _build_nc()
    nc = _NC_CACHE["nc"]

    shards = np.split(u0, N_CORES, axis=0)
    in_maps = [{"u": s, **consts} for s in shards]
    res = run_bass_kernel_spmd(
        nc, in_maps, core_ids=list(range(N_CORES)), trace=TRACE
    )
    LAST_RESULTS = res
    return np.concatenate([r["out"] for r in res.results], axis=0)


# revision 31
# speedup vs baseline: 1.4272x; 1.4272x over previous
"""Trainium2 Bass kernel for nn_HeatEquation1D.

The reference applies a fixed 62x62 Crank-Nicolson step matrix 100 times to
u0[:, 1:-1] via lax.scan, then zero-pads the boundary columns.  Algebraically
that whole scan is a single matmul:

    out = u0 @ W64,   W64[1:63, 1:63] = (step_matrix^100).T,  zero elsewhere

(the zero rows/cols of W64 implement both the dropped boundary inputs and the
zero Dirichlet boundary outputs).  W64 is computed on the host in float64.

Device kernel (per core, pure data parallel over 8 cores):
  - u shard (65536, 64) f32 moves in SUPER_ROWS-row super-blocks; each
    partition holds SUPER_ROWS/128 consecutive rows, so one dma_start is a
    single large contiguous chunk per partition (amortizes the fixed
    per-DMA overheads).  The input DMA is SWDGE and casts f32->bf16
    inline, so every PE operand is bf16 (1 cyc/row + fast weight load).
  - Compute runs in 2048-row groups (fits PSUM):
      * 8 PE transposes (bf16): X chunk [128, 2rows x 64feat] -> T1 in PSUM.
      * 1 DVE copy PSUM->SBUF bf16 (all-2-byte operands -> DVE 2x mode).
      * 8 PE matmuls: stationary = T1 chunk, moving = BD where
        BD = block_diag(W64, W64) (128x128) in bf16.  Because T1 chunk
        columns are (row-pair, feature) interleaved, BD applies W64 to each
        row of the pair and the result lands batch-major in fp32 PSUM --
        no second transpose.
      * 1 ACT copy PSUM->SBUF fp32 (output copy on the Scalar engine so the
        two mandatory PSUM evacuations ride different engines).
  - One contiguous HWDGE dma_start out per super-block.

Measured 8-core-concurrent on trn2: ~108 us/core vs a 95 us pure-DMA
floor (33.55 MB at the ~352 GB/s HBM-per-core limit); PE/DVE/ACT hide
underneath.  Numerics: single bf16 rounding of data and matrix,
rel_err ~2.3e-3 (tolerance 2e-2).
"""

import numpy as np

BATCH = 524288
NX = 64
N_INNER = NX - 2
NUM_STEPS = 100
N_CORES = 8
ROWS_PER_CORE = BATCH // N_CORES           # 65536
P = 128

SUPER_ROWS = 16384                         # rows per DMA super-block (4 MiB)
GROUP_ROWS = 2048                          # rows per compute group (PSUM bound)

# Set by callers that want a profile; results object stashed in LAST_RESULTS.
TRACE = False
LAST_RESULTS = None

_NC_CACHE = {}


def _build_nc(reps=1, dma_only=False, super_rows=SUPER_ROWS,
              group_rows=GROUP_ROWS, split_bd=False, cast="dma",
              xbufs=3, psbufs=2, ycopy="act"):
    """reps>1 wraps the whole pass in a hardware For_i loop (for benching).

    cast: how u f32 becomes bf16 before the PE transposes (bf16 operands
    run 1 cyc/row on the PE and get fast weight load):
      "engine" - plain HWDGE f32 input DMA, GPSIMD casts each group's
                 slice to bf16 (measured terrible: Q7 copy is slow).
      "dma"    - SWDGE input DMA casts inline (measured ~11% below DMA
                 line rate at 8 cores -> ~8.5 us/pass penalty).
      "fp32r"  - plain HWDGE f32 input DMA at full line rate; transposes
                 run in float32r (same bits as f32, 1.5 cyc/row, FWL
                 eligible); the bf16 cast rides the mandatory DVE copy.
      "none"   - stay f32 into the transposes (PE 2 cyc/row, no FWL).
    """
    from concourse import bacc, mybir
    from concourse.tile import TileContext

    nc = bacc.Bacc("TRN2", target_bir_lowering=False, debug=False)
    f32 = mybir.dt.float32
    bf16 = mybir.dt.bfloat16
    f32r = mybir.dt.float32r
    if dma_only:
        cast = "none"
    assert cast in ("engine", "dma", "fp32r", "none")
    # dtype of the DMA-in tile / dtype feeding the transposes
    x_dt = {"dma": bf16, "fp32r": f32r}.get(cast, f32)
    t_dt = {"dma": bf16, "engine": bf16, "fp32r": f32r}.get(cast, f32)
    in_dma = nc.gpsimd.dma_start if cast == "dma" else nc.sync.dma_start

    n_super = ROWS_PER_CORE // super_rows
    rpp_s = super_rows // P                 # rows per partition per super tile
    groups = super_rows // group_rows       # compute groups per super-block
    rpp_g = group_rows // P                 # rows per partition per group
    ch = rpp_g // 2                         # chunks of 128 cols per group

    u_dt = f32r if cast == "fp32r" else f32   # f32r is bit-identical to f32
    u = nc.dram_tensor("u", [ROWS_PER_CORE, NX], u_dt, kind="ExternalInput")
    bdh_d = nc.dram_tensor("bd_hi", [P, P], bf16, kind="ExternalInput")
    bdl_d = nc.dram_tensor("bd_lo", [P, P], bf16, kind="ExternalInput")
    id_d = nc.dram_tensor("ident", [P, P], t_dt, kind="ExternalInput")
    out = nc.dram_tensor("out", [ROWS_PER_CORE, NX], f32, kind="ExternalOutput")

    u_r = u.rearrange("(nb p r) f -> nb p r f", p=P, r=rpp_s)
    out_r = out.rearrange("(nb p r) f -> nb p r f", p=P, r=rpp_s)

    with TileContext(nc) as tc:
        with (
            tc.tile_pool(name="consts", bufs=1) as cpool,
            tc.tile_pool(name="xin", bufs=xbufs) as xpool,
            tc.tile_pool(name="xb16", bufs=3) as bpool,
            tc.tile_pool(name="t1s", bufs=3) as tpool,
            tc.tile_pool(name="yout", bufs=3) as ypool,
            tc.tile_pool(name="ps_t", bufs=psbufs, space="PSUM") as pst,
            tc.tile_pool(name="ps_y", bufs=2, space="PSUM") as psy,
        ):
            bdh_s = cpool.tile([P, P], bf16)
            bdl_s = cpool.tile([P, P], bf16)
            id_s = cpool.tile([P, P], t_dt)
            nc.sync.dma_start(out=bdh_s[:], in_=bdh_d[:])
            nc.sync.dma_start(out=bdl_s[:], in_=bdl_d[:])
            nc.sync.dma_start(out=id_s[:], in_=id_d[:])

            def one_pass():
                for nb in range(n_super):
                    x = xpool.tile([P, rpp_s, NX], x_dt)
                    in_dma(out=x[:], in_=u_r[nb])

                    if dma_only:
                        nc.sync.dma_start(out=out_r[nb], in_=x[:])
                        continue

                    y = ypool.tile([P, rpp_s, NX], f32)
                    for g in range(groups):
                        r0 = g * rpp_g
                        if cast == "engine":
                            src = bpool.tile([P, rpp_g, NX], bf16)
                            nc.gpsimd.tensor_copy(
                                out=src[:], in_=x[:, r0 : r0 + rpp_g, :]
                            )
                            s0 = 0
                        else:
                            src, s0 = x, r0
                        t1p = pst.tile([P, ch, P], t_dt)
                        for c in range(ch):
                            nc.tensor.transpose(
                                t1p[:, c],
                                src[:, s0 + 2 * c : s0 + 2 * c + 2, :],
                                id_s[:],
                            )
                        t1s = tpool.tile([P, ch, P], bf16)
                        nc.vector.tensor_copy(out=t1s[:], in_=t1p[:])

                        yp = psy.tile([P, ch, P], f32)
                        for c in range(ch):
                            nc.tensor.matmul(
                                yp[:, c], t1s[:, c], bdh_s[:],
                                start=True, stop=not split_bd,
                            )
                            if split_bd:
                                nc.tensor.matmul(
                                    yp[:, c], t1s[:, c], bdl_s[:],
                                    start=False, stop=True,
                                )
                        if ycopy == "dve":
                            # Measured worse (120 vs 108 us at 8 cores):
                            # serializing both PSUM evacuations on DVE
                            # loses more than ACT's slower copy costs.
                            nc.vector.tensor_copy(
                                out=y[:, r0 : r0 + rpp_g, :], in_=yp[:]
                            )
                        else:
                            nc.scalar.copy(
                                out=y[:, r0 : r0 + rpp_g, :], in_=yp[:]
                            )
                    nc.sync.dma_start(out=out_r[nb], in_=y[:])

            if reps == 1:
                one_pass()
            else:
                with tc.For_i(0, reps, 1,
                              hint_engines=(mybir.EngineType.PE,)):
                    one_pass()

    nc.compile()
    return nc


def _host_matrices(step_matrix):
    import ml_dtypes

    m = np.asarray(step_matrix, dtype=np.float64)
    w_inner = np.linalg.matrix_power(m, NUM_STEPS).T  # right-multiplier, f64
    w64 = np.zeros((NX, NX), dtype=np.float64)
    w64[1 : NX - 1, 1 : NX - 1] = w_inner
    bd = np.zeros((P, P), dtype=np.float64)
    bd[:NX, :NX] = w64
    bd[NX:, NX:] = w64
    bd_hi = bd.astype(ml_dtypes.bfloat16)
    bd_lo = (bd - bd_hi.astype(np.float64)).astype(ml_dtypes.bfloat16)
    return bd_hi, bd_lo


def _const_inputs(step_matrix, cast="dma"):
    import ml_dtypes

    bd_hi, bd_lo = _host_matrices(step_matrix)
    id_dt = ml_dtypes.bfloat16 if cast in ("dma", "engine") else np.float32
    ident = np.eye(P, dtype=id_dt)
    return {"bd_hi": bd_hi, "bd_lo": bd_lo, "ident": ident}


def kernel(u0, step_matrix):
    global LAST_RESULTS
    from concourse.bass_utils import run_bass_kernel_spmd

    u0 = np.ascontiguousarray(np.asarray(u0, dtype=np.float32))
    assert u0.shape == (BATCH, NX), u0.shape

    consts = _const_inputs(step_matrix)

    if "nc" not in _NC_CACHE:
        _NC_CACHE["nc"] = _build_nc()
    nc = _NC_CACHE["nc"]

    shards = np.split(u0, N_CORES, axis=0)
    in_maps = [{"u": s, **consts} for s in shards]
    res = run_bass_kernel_spmd(
        nc, in_maps, core_ids=list(range(N_CORES)), trace=TRACE
    )
    LAST_RESULTS = res
    return np.concatenate([r["out"] for r in res.results], axis=0)
